# revision 14
# baseline (speedup 1.0000x reference)
"""Trainium2 Bass kernel for nn_DevelopmentalLayerV51 (moe_routing).

kernel(**inputs) takes the FULL unsharded inputs (as reference.setup_inputs)
and returns the full (h, dh) tuple of np.float32 arrays.

Sharding: data-parallel over the B*S=8192 tokens across 8 cores (core c owns
batch c//2, sequence half c%2 -> 1024 tokens). Top-k routing uses only
scores[0], so it is decided on host; only the selected blocks' weights are
shipped (replicated, bf16). On device all activations are feature-major
[D, tokens] so every matmul is transpose-free (contraction dim in
partitions for both operands); LayerNorm/softmax partition-reductions use
ones-matmuls on the PE. Attention K/V are AllGathered within the core pair
sharing a batch; delayed = mean_S(disembodied) via a tiny pair AllReduce.
Residual streams stay fp32 and round-trip through DRAM between phases so
SBUF only ever holds one phase's working set.
"""
import sys

sys.path.insert(0, "/opt/trn_rl_repo")

import contextlib

import numpy as np
import ml_dtypes

import concourse.bass as bass
import concourse.tile as tile
from concourse import bacc, mybir

DIM = 1024
NBLK = 8
NHEAD = 4
HD = DIM // NHEAD          # 256
B, S = 4, 2048
NCORES = 8
TLOC = (B * S) // NCORES   # 1024 tokens per core
P = 128
DC = DIM // P              # 8 feature tiles
DT = TLOC // P             # 8 token tiles
FF = 4 * DIM               # 4096
FC = FF // P               # 32
F32 = mybir.dt.float32
BF16 = mybir.dt.bfloat16
AF = mybir.ActivationFunctionType
AX = mybir.AxisListType
ALU = mybir.AluOpType

PAIRS = [[0, 1], [2, 3], [4, 5], [6, 7]]
QS = [slice(0, 512), slice(512, 1024)]


def build_program(kk):
    nc = bacc.Bacc("TRN2", target_bir_lowering=False, debug=False,
                   num_devices=NCORES)
    dp = nc.declare_dram_parameter

    x_emb = dp("x_emb", [DIM, TLOC], F32, isOutput=False)
    x_dis = dp("x_dis", [DIM, TLOC], F32, isOutput=False)
    common = dp("common", [P, DC, 2], F32, isOutput=False)  # esc, s05
    e_w1 = [dp(f"e{i}_w1", [DIM, DIM], BF16, isOutput=False) for i in range(kk)]
    e_w2 = [dp(f"e{i}_w2", [DIM, DIM], BF16, isOutput=False) for i in range(kk)]
    # cols: ln_s, ln_b, b1, esc*b2
    e_vec = [dp(f"e{i}_vec", [P, DC, 4], F32, isOutput=False) for i in range(kk)]
    d_wqk = [dp(f"d{i}_wqk", [DIM, 2 * DIM], BF16, isOutput=False) for i in range(kk)]
    d_wv = [dp(f"d{i}_wv", [DIM, DIM], BF16, isOutput=False) for i in range(kk)]
    d_wout = [dp(f"d{i}_wout", [DIM, DIM], BF16, isOutput=False) for i in range(kk)]
    d_ff1 = [dp(f"d{i}_ff1", [DIM, FF], BF16, isOutput=False) for i in range(kk)]
    d_ff2 = [dp(f"d{i}_ff2", [FF, DIM], BF16, isOutput=False) for i in range(kk)]
    # cols: ln1_s, ln1_b, ln2_s, ln2_b, bq/16, bk, b_out, 0.5*b_ff2
    d_vec = [dp(f"d{i}_vec", [P, DC, 8], F32, isOutput=False) for i in range(kk)]
    d_ff1b = [dp(f"d{i}_ff1b", [P, FC], F32, isOutput=False) for i in range(kk)]
    d_bv = [dp(f"d{i}_bv", [1, DIM], BF16, isOutput=False) for i in range(kk)]
    out_e = dp("out_e", [DIM, TLOC], F32, isOutput=True)
    out_d = dp("out_d", [DIM, TLOC], F32, isOutput=True)

    dt_ = nc.dram_tensor
    cc_del_in = dt_("cc_del_in", [P, DC], F32)
    cc_del_out = dt_("cc_del_out", [P, DC], F32)
    e_mid = [dt_(f"e_mid{i}", [DIM, TLOC], F32) for i in range(max(kk - 1, 0))]
    d_mid = [dt_(f"d_mid{i}", [DIM, TLOC], F32) for i in range(max(kk - 1, 0))]
    h_mid = [dt_(f"h_mid{i}", [DIM, TLOC], F32) for i in range(kk)]
    xn2_d = [dt_(f"xn2_d{i}", [DIM, TLOC], BF16) for i in range(kk)]
    q_loc = [dt_(f"q_loc{i}", [DIM, TLOC], BF16) for i in range(kk)]
    o_loc = [dt_(f"o_loc{i}", [DIM, TLOC], BF16) for i in range(kk)]
    k_loc = [dt_(f"k_loc{i}", [DIM, TLOC], BF16) for i in range(kk)]
    v_loc = [dt_(f"v_loc{i}", [DIM, TLOC], BF16) for i in range(kk)]
    k_full = [dt_(f"k_full{i}", [2, DIM, TLOC], BF16) for i in range(kk)]
    v_full = [dt_(f"v_full{i}", [2, DIM, TLOC], BF16) for i in range(kk)]

    e_src = [x_emb] + e_mid
    e_dst = e_mid + [out_e]
    d_src = [x_dis] + d_mid
    d_dst = d_mid + [out_d]

    with tile.TileContext(nc, pool_alloc_mode="queue") as tc, \
         contextlib.ExitStack() as octx:
        singles = octx.enter_context(tc.tile_pool(name="singles", bufs=1))
        stats = octx.enter_context(tc.tile_pool(name="stats", bufs=1))
        psum = octx.enter_context(tc.tile_pool(name="psum", bufs=4,
                                               space="PSUM"))

        def ps_tile(name):
            return psum.tile([P, TLOC], F32, tag="ps", name=name)

        def ps1_tile(name):
            return psum.tile([1, TLOC], F32, tag="ps", name=name)

        ones128_bf = singles.tile([P, 1], BF16)
        nc.vector.memset(ones128_bf, 1.0)
        ones1_f32 = singles.tile([1, P], F32)
        nc.vector.memset(ones1_f32, 1.0)
        ones1_bf = singles.tile([1, P], BF16)
        nc.vector.memset(ones1_bf, 1.0)

        eps_sb = singles.tile([P, 1], F32)
        nc.vector.memset(eps_sb, 1e-5)
        common_sb = singles.tile([P, DC, 2], F32)
        nc.sync.dma_start(out=common_sb, in_=common[:, :, :])
        evec_sb = []
        dvec_sb = []
        ff1b_sb = []
        bv_sb = []
        for i in range(kk):
            t = singles.tile([P, DC, 4], F32, name=f"evec{i}", tag=f"evec{i}")
            nc.sync.dma_start(out=t, in_=e_vec[i][:, :, :])
            evec_sb.append(t)
            t = singles.tile([P, DC, 8], F32, name=f"dvec{i}", tag=f"dvec{i}")
            nc.sync.dma_start(out=t, in_=d_vec[i][:, :, :])
            dvec_sb.append(t)
            t = singles.tile([P, FC], F32, name=f"ff1b{i}", tag=f"ff1b{i}")
            nc.sync.dma_start(out=t, in_=d_ff1b[i][:, :])
            ff1b_sb.append(t)
            t = singles.tile([1, DIM], BF16, name=f"bv{i}", tag=f"bv{i}")
            nc.sync.dma_start(out=t, in_=d_bv[i][:, :])
            bv_sb.append(t)

        def act(out, in_, func=AF.Copy, bias=0.0, scale=1.0):
            if func == AF.Copy and not isinstance(bias, float):
                func = AF.Identity  # Copy rejects AP bias; Identity is affine
            nc.scalar.activation(out=out, in_=in_, func=func, bias=bias,
                                 scale=scale)

        def w_cols(pool, w_dram, kc, m0, msz, tag):
            """[kc*128, *] bf16 DRAM weight -> sbuf [P, kc, msz] col block."""
            t = pool.tile([P, kc, msz], BF16, tag=tag, name=f"w_{tag}")
            src = w_dram.rearrange("(k p) m -> p k m", p=P)
            nc.sync.dma_start(out=t, in_=src[:, 0:kc, m0:m0 + msz])
            return t

        def mm_feature(wpool, w_dram, rhs_tiles, n_out, evict, wtag):
            """psum[m] = W[:, m*128:(m+1)*128].T @ rhs, evict(m, ps)."""
            kc = len(rhs_tiles)
            for m in range(n_out):
                wt = w_cols(wpool, w_dram, kc, m * P, P, wtag)
                ps = ps_tile(f"ps_{wtag}_{m}")
                last = 2 * kc - 2
                for i, d in enumerate(range(kc)):
                    for q in range(2):
                        nc.tensor.matmul(
                            ps[:, QS[q]], wt[:, d, :], rhs_tiles[d][:, QS[q]],
                            start=(i == 0), stop=(2 * i >= last))
                evict(m, ps)

        def layer_norm(ph, src_dram, vec, s_col, b_col, name):
            """src (DRAM [DIM, TLOC] fp32) -> 8 bf16 [P, TLOC] sbuf tiles."""
            mean_ps = ps1_tile(f"lnm_{name}")
            sq_ps = ps1_tile(f"lnsq_{name}")
            xb_l = []
            for d in range(DC):
                xs = ph.tile([P, TLOC], F32, tag="ln_xs", name=f"lnxs_{name}")
                nc.sync.dma_start(out=xs, in_=src_dram[d * P:(d + 1) * P, :])
                xb = ph.tile([P, TLOC], BF16, tag="ln_xb", bufs=DC + 1,
                             name=f"lnxb_{name}")
                nc.vector.tensor_copy(xb, xs)
                xb_l.append(xb)
                sq = ph.tile([P, TLOC], BF16, tag="ln_sq", name=f"lnq_{name}")
                nc.vector.tensor_mul(sq, xb, xb)
                for q in range(2):
                    nc.tensor.matmul(mean_ps[:, QS[q]], ones128_bf, xb[:, QS[q]],
                                     start=(d == 0), stop=(d == DC - 1))
                    nc.tensor.matmul(sq_ps[:, QS[q]], ones128_bf, sq[:, QS[q]],
                                     start=(d == 0), stop=(d == DC - 1))
            nm = stats.tile([1, TLOC], F32, tag="st1", bufs=4, name=f"nm_{name}")
            act(nm, mean_ps, AF.Copy, scale=-1.0 / DIM)
            msq = stats.tile([1, TLOC], F32, tag="st1", bufs=4, name=f"msq_{name}")
            act(msq, sq_ps, AF.Copy, scale=1.0 / DIM)
            var = stats.tile([1, TLOC], F32, tag="st1", bufs=4, name=f"var_{name}")
            nc.vector.tensor_mul(var, nm, nm)
            nc.vector.tensor_sub(var, msq, var)
            rstd = stats.tile([1, 2, TLOC], F32, tag="st2", name=f"rstd_{name}")
            std = stats.tile([1, TLOC], F32, tag="st1", bufs=4,
                             name=f"std_{name}")
            act(std, var, AF.Sqrt, bias=eps_sb[:1])
            nc.vector.reciprocal(rstd[:, 0, :], std)
            nc.vector.tensor_mul(rstd[:, 1, :], nm, rstd[:, 0, :])
            bc = ph.tile([P, 2, TLOC], F32, tag="ln_bc", name=f"bc_{name}")
            for j in range(2):
                bc_ps = ps_tile(f"lnbc_{name}{j}")
                for q in range(2):
                    nc.tensor.matmul(bc_ps[:, QS[q]], ones1_f32,
                                     rstd[:, j, QS[q]], start=True, stop=True)
                act(bc[:, j, :], bc_ps)
            out_tiles = []
            for d in range(DC):
                t1 = ph.tile([P, TLOC], F32, tag="ln_t1", name=f"t1_{name}")
                nc.vector.tensor_mul(t1, xb_l[d], bc[:, 0, :])
                nc.vector.tensor_add(t1, t1, bc[:, 1, :])
                xn = ph.tile([P, TLOC], BF16, tag="ln_xn", bufs=DC + 1,
                             name=f"xn_{name}")
                act(xn, t1, AF.Copy, bias=vec[:, d, b_col:b_col + 1],
                    scale=vec[:, d, s_col:s_col + 1])
                out_tiles.append(xn)
            return out_tiles

        # delayed = mean_S(x_dis): partial sums ride d0's LN1 stream, then
        # a pair AllReduce. dacc lives in singles; finalized in emit_d_qkv(0).
        delayed_sb = singles.tile([P, DC], F32)
        dacc = singles.tile([P, DC], F32, name="dacc")

        def delayed_hook(d, xsrc):
            r = stats.tile([P, 1], F32, tag="dred", bufs=3, name="dred")
            nc.vector.tensor_reduce(r, xsrc, axis=AX.X, op=ALU.add)
            nc.vector.tensor_scalar_mul(dacc[:, d:d + 1], r, 1.0 / S)

        def delayed_finalize():
            nc.sync.dma_start(out=cc_del_in[:, :], in_=dacc)
            nc.gpsimd.collective_compute(
                "AllReduce", ALU.add, replica_groups=PAIRS,
                ins=[cc_del_in[:, :]], outs=[cc_del_out[:, :]])
            nc.sync.dma_start(out=delayed_sb, in_=cc_del_out[:, :])

        # ------------ embodied path ----------------------------------------
        for i in range(kk):
            vec = evec_sb[i]
            with tc.tile_pool(name=f"ph_e{i}", bufs=1) as ph:
                xn = layer_norm(ph, e_src[i], vec, 0, 1, f"e{i}")
                h1 = []

                def ev_tanh(m, ps, vec=vec, ph=ph, h1=h1):
                    t = ph.tile([P, TLOC], BF16, tag="h1", bufs=DC + 1,
                                name="h1")
                    act(t, ps, AF.Tanh, bias=vec[:, m, 2:3])
                    h1.append(t)
                mm_feature(ph, e_w1[i], xn, DC, ev_tanh, "w8a")

                def ev_e2(m, ps, vec=vec, ph=ph, i=i):
                    y = ph.tile([P, TLOC], F32, tag="y", bufs=3, name="y")
                    act(y, ps, AF.Copy, bias=vec[:, m, 3:4],
                        scale=common_sb[:, m, 0:1])
                    xs = ph.tile([P, TLOC], F32, tag="xadd", bufs=3, name="xs")
                    nc.sync.dma_start(out=xs,
                                      in_=e_src[i][m * P:(m + 1) * P, :])
                    xo = ph.tile([P, TLOC], F32, tag="xout", bufs=3, name="xo")
                    nc.vector.tensor_add(xo, xs, y)
                    nc.sync.dma_start(out=e_dst[i][m * P:(m + 1) * P, :],
                                      in_=xo)
                mm_feature(ph, e_w2[i], h1, DC, ev_e2, "w8b")

        # ------------ disembodied path -------------------------------------
        for i in range(kk):
            vec = dvec_sb[i]
            # --- phase D1: ln1 + qk + v projections ---
            with tc.tile_pool(name=f"ph_d{i}a", bufs=1) as ph:
                xn1 = layer_norm(ph, d_src[i], vec, 0, 1, f"d{i}l1",
                                 xs_hook=(delayed_hook if i == 0 else None))
                if i == 0:
                    delayed_finalize()

                def ev_qk(m, ps, vec=vec, ph=ph, i=i):
                    if m < DC:
                        t = ph.tile([P, TLOC], BF16, tag="qk_ev", bufs=3,
                                    name="qev")
                        act(t, ps, AF.Copy, bias=vec[:, m, 4:5],
                            scale=1.0 / 16.0)
                        nc.sync.dma_start(
                            out=q_loc[i][m * P:(m + 1) * P, :], in_=t)
                    else:
                        t = ph.tile([P, TLOC], BF16, tag="qk_ev", bufs=3,
                                    name="kev")
                        act(t, ps, AF.Copy, bias=vec[:, m - DC, 5:6])
                        nc.sync.dma_start(
                            out=kv_loc[i][0, (m - DC) * P:(m - DC + 1) * P, :],
                            in_=t)
                mm_feature(ph, d_wqk[i], xn1, 2 * DC, ev_qk, "w8a",
                           m_range=range(DC, 2 * DC))

                nc.gpsimd.collective_compute(
                    "AllGather", ALU.bypass, replica_groups=PAIRS,
                    ins=[k_loc[i][:, :]], outs=[k_full[i][:, :, :]])

                # V token-major: lhsT = xn1 slices, rhs = Wv row-chunks
                wv_rows = []
                wv_src = d_wv[i].rearrange("(k p) m -> p k m", p=P)
                for d in range(DC):
                    t = ph.tile([P, DIM], BF16, tag="wv_row", bufs=DC,
                                name="wvr")
                    nc.gpsimd.dma_start(out=t, in_=wv_src[:, d, :])
                    wv_rows.append(t)
                for m in range(DT):
                    ps = ps_tile(f"ps_v{m}")
                    for d in range(DC):
                        for q in range(2):
                            nc.tensor.matmul(
                                ps[:, QS[q]], xn1[d][:, m * P:(m + 1) * P],
                                wv_rows[d][:, QS[q]],
                                start=(d == 0), stop=False)
                    for q in range(2):
                        nc.tensor.matmul(ps[:, QS[q]], ones1_bf,
                                         bv_sb[i][:, QS[q]],
                                         start=False, stop=(q == 1))
                    vt = ph.tile([P, TLOC], BF16, tag="qk_ev", bufs=3,
                                 name="vev")
                    nc.vector.tensor_copy(vt, ps)
                    nc.sync.dma_start(out=kv_loc[i][1, m * P:(m + 1) * P, :],
                                      in_=vt)

            nc.gpsimd.collective_compute(
                "AllGather", ALU.bypass, replica_groups=PAIRS,
                ins=[v_loc[i][:, :]], outs=[v_full[i][:, :, :]])

            # --- phase D2: attention ---
            with tc.tile_pool(name=f"ph_d{i}b", bufs=1) as ph:
                v_res = []
                for kt in range(2 * DT):
                    r, m = kt // DT, kt % DT
                    t = ph.tile([P, DIM], BF16, tag="v_res", bufs=2 * DT,
                                name="vres")
                    nc.gpsimd.dma_start(
                        out=t, in_=v_full[i][r, m * P:(m + 1) * P, :])
                    v_res.append(t)
                for h in range(NHEAD):
                    qh = []
                    for s in range(2):
                        t = ph.tile([P, TLOC], BF16, tag="qh", bufs=4,
                                    name="qh")
                        fr = h * HD + s * P
                        nc.gpsimd.dma_start(out=t, in_=q_loc[i][fr:fr + P, :])
                        qh.append(t)
                    av_ps = [ps_tile(f"av{h}_{s2}") for s2 in range(2)]
                    den_ps = ps1_tile(f"den{h}")
                    for kt in range(2 * DT):
                        r, c = kt // DT, (kt % DT) * P
                        sc_ps = ps_tile(f"sc{h}_{kt}")
                        for s in range(2):
                            kT = ph.tile([P, P], BF16, tag="kT", bufs=4,
                                         name="kT")
                            fr = h * HD + s * P
                            nc.sync.dma_start(
                                out=kT,
                                in_=kv_full[i][r, 0, fr:fr + P, c:c + P])
                            for q in range(2):
                                nc.tensor.matmul(sc_ps[:, QS[q]], kT,
                                                 qh[s][:, QS[q]],
                                                 start=(s == 0), stop=(s == 1))
                        et = ph.tile([P, TLOC], BF16, tag="expT", bufs=4,
                                     name="et")
                        act(et, sc_ps, AF.Exp)
                        first, last = kt == 0, kt == 2 * DT - 1
                        for q in range(2):
                            nc.tensor.matmul(den_ps[:, QS[q]], ones128_bf,
                                             et[:, QS[q]], start=first,
                                             stop=last)
                            for s2 in range(2):
                                fr = h * HD + s2 * P
                                nc.tensor.matmul(
                                    av_ps[s2][:, QS[q]],
                                    v_res[kt][:, fr:fr + P], et[:, QS[q]],
                                    start=first, stop=last)
                    recip = stats.tile([1, TLOC], F32, tag="st1", bufs=4,
                                       name="recip")
                    nc.vector.reciprocal(recip, den_ps)
                    rb_ps = ps_tile(f"rb{h}")
                    for q in range(2):
                        nc.tensor.matmul(rb_ps[:, QS[q]], ones1_f32,
                                         recip[:, QS[q]], start=True,
                                         stop=True)
                    rb = ph.tile([P, TLOC], F32, tag="rb", bufs=2, name="rb")
                    act(rb, rb_ps)
                    for s2 in range(2):
                        ot = ph.tile([P, TLOC], BF16, tag="ot", bufs=3,
                                     name="ot")
                        nc.vector.tensor_mul(ot, av_ps[s2], rb)
                        fr = h * HD + s2 * P
                        nc.sync.dma_start(out=o_loc[i][fr:fr + P, :], in_=ot)

            # --- phase D3: out projection + residual ---
            bias_d = singles.tile([P, DC], F32, tag=f"biasd{i}",
                                  name=f"biasd{i}")
            nc.vector.tensor_scalar_mul(bias_d, delayed_sb, 0.3)
            nc.vector.tensor_add(bias_d, bias_d, vec[:, :, 6])
            nc.vector.tensor_mul(bias_d, bias_d, common_sb[:, :, 1])
            with tc.tile_pool(name=f"ph_d{i}c", bufs=1) as ph:
                o_res = []
                for d in range(DC):
                    t = ph.tile([P, TLOC], BF16, tag="o_res", bufs=DC,
                                name="ores")
                    nc.sync.dma_start(out=t, in_=o_loc[i][d * P:(d + 1) * P, :])
                    o_res.append(t)

                def ev_out(m, ps, vec=vec, ph=ph, i=i):
                    y = ph.tile([P, TLOC], F32, tag="y", bufs=3, name="y")
                    act(y, ps, AF.Copy, bias=bias_d[:, m:m + 1],
                        scale=common_sb[:, m, 1:2])
                    xs = ph.tile([P, TLOC], F32, tag="xadd", bufs=3, name="xs")
                    nc.sync.dma_start(out=xs,
                                      in_=d_src[i][m * P:(m + 1) * P, :])
                    xo = ph.tile([P, TLOC], F32, tag="xout", bufs=3, name="xo")
                    nc.vector.tensor_add(xo, xs, y)
                    nc.sync.dma_start(out=h_mid[i][m * P:(m + 1) * P, :],
                                      in_=xo)
                mm_feature(ph, d_wout[i], o_res, DC, ev_out, "w8b")

            # --- phase D4a: ln2 -> xn2 to DRAM (bf16) ---
            with tc.tile_pool(name=f"ph_d{i}d", bufs=1) as ph:
                xn2 = layer_norm(ph, h_mid[i], vec, 2, 3, f"d{i}l2")
                for d in range(DC):
                    nc.sync.dma_start(out=xn2_d[i][d * P:(d + 1) * P, :],
                                      in_=xn2[d])

            # --- phase D4b: FFN ---
            with tc.tile_pool(name=f"ph_d{i}e", bufs=1) as ph:
                xn2r = []
                for d in range(DC):
                    t = ph.tile([P, TLOC], BF16, tag="xn2r", bufs=DC,
                                name="xn2r")
                    nc.sync.dma_start(out=t,
                                      in_=xn2_d[i][d * P:(d + 1) * P, :])
                    xn2r.append(t)
                g_big = ph.tile([P, FC, TLOC], BF16, tag="g_big", name="g")

                def ev_g(m, ps, i=i):
                    act(g_big[:, m, :], ps, AF.Gelu, bias=ff1b_sb[i][:, m:m + 1])
                mm_feature(ph, d_ff1[i], xn2r, FC, ev_g, "w8a")

                g_tiles = [g_big[:, m, :] for m in range(FC)]

                def ev_f2(m, ps, vec=vec, ph=ph, i=i):
                    y = ph.tile([P, TLOC], F32, tag="y", bufs=3, name="y")
                    act(y, ps, AF.Copy, bias=vec[:, m, 7:8], scale=0.5)
                    xs = ph.tile([P, TLOC], F32, tag="xadd", bufs=3, name="xs")
                    nc.sync.dma_start(out=xs,
                                      in_=h_mid[i][m * P:(m + 1) * P, :])
                    xo = ph.tile([P, TLOC], F32, tag="xout", bufs=3, name="xo")
                    nc.vector.tensor_add(xo, xs, y)
                    nc.sync.dma_start(out=d_dst[i][m * P:(m + 1) * P, :],
                                      in_=xo)
                mm_feature(ph, d_ff2[i], g_tiles, DC, ev_f2, "w32")

    nc.compile()
    return nc


# ---------------------------------------------------------------------------
# host side
# ---------------------------------------------------------------------------
_prog_cache = {}


def _pack_pvec(v):
    """[1024] -> [128, 8]: out[p, i] = v[i*128 + p]."""
    return np.ascontiguousarray(np.asarray(v, np.float32).reshape(-1, P).T)


def _bf(x):
    return np.ascontiguousarray(x).astype(ml_dtypes.bfloat16)


def kernel(**inputs):
    inp = {k: np.asarray(v) for k, v in inputs.items()}
    kk = min(int(inp["max_active_blocks"]), NBLK)
    emb = inp["embodied_input"].astype(np.float32)
    dis = inp["disembodied_input"].astype(np.float32)
    tf = inp["torsion_field"].astype(np.float32)

    if kk == 0:
        return emb.copy(), dis.copy()

    def sigmoid(x):
        return 1.0 / (1.0 + np.exp(-x))

    esc_s = sigmoid(emb[0].mean(axis=0) @ inp["esel_w"].T + inp["esel_b"])
    etop = np.argsort(-esc_s, kind="stable")[:kk]
    dsc_s = sigmoid(dis[0].mean(axis=0) @ inp["dsel_w"].T + inp["dsel_b"])
    dtop = np.argsort(-dsc_s, kind="stable")[:kk]

    if kk not in _prog_cache:
        _prog_cache[kk] = build_program(kk)
    nc = _prog_cache[kk]

    wmap = {}
    for i, idx in enumerate(etop):
        wmap[f"e{i}_w1"] = _bf(inp["e_w1"][idx].T)
        wmap[f"e{i}_w2"] = _bf(inp["e_w2"][idx].T)
    for i, idx in enumerate(dtop):
        qkv_w = inp["d_qkv_w"][idx]  # [3072, 1024]
        qkv_b = inp["d_qkv_b"][idx]
        wmap[f"d{i}_wqk"] = _bf(qkv_w[:2 * DIM].T)
        wmap[f"d{i}_wv"] = _bf(qkv_w[2 * DIM:].T)
        wmap[f"d{i}_wout"] = _bf(inp["d_out_w"][idx].T)
        wmap[f"d{i}_ff1"] = _bf(inp["d_ff1_w"][idx].T)
        wmap[f"d{i}_ff2"] = _bf(inp["d_ff2_w"][idx].T)
        wmap[f"d{i}_vec"] = np.stack([
            _pack_pvec(inp["d_ln1_s"][idx]),
            _pack_pvec(inp["d_ln1_b"][idx]),
            _pack_pvec(inp["d_ln2_s"][idx]),
            _pack_pvec(inp["d_ln2_b"][idx]),
            _pack_pvec(qkv_b[:DIM] / 16.0),
            _pack_pvec(qkv_b[DIM:2 * DIM]),
            _pack_pvec(inp["d_out_b"][idx]),
            _pack_pvec(0.5 * inp["d_ff2_b"][idx]),
        ], axis=-1)
        wmap[f"d{i}_ff1b"] = _pack_pvec(inp["d_ff1_b"][idx])
        wmap[f"d{i}_bv"] = _bf(qkv_b[2 * DIM:].reshape(1, DIM))

    in_maps = []
    for c in range(NCORES):
        b, half = c // 2, c % 2
        tsl = slice(half * TLOC, (half + 1) * TLOC)
        m = dict(wmap)
        m["x_emb"] = np.ascontiguousarray(emb[b, tsl].T)
        m["x_dis"] = np.ascontiguousarray(dis[b, tsl].T)
        esc_v = 0.3 * (1.0 + 0.1 * tf[b])
        s05_v = 0.5 * (1.0 + 0.05 * tf[b])
        m["common"] = np.stack([_pack_pvec(esc_v), _pack_pvec(s05_v)], axis=-1)
        for i, idx in enumerate(etop):
            m[f"e{i}_vec"] = np.stack([
                _pack_pvec(inp["e_ln_s"][idx]),
                _pack_pvec(inp["e_ln_b"][idx]),
                _pack_pvec(inp["e_b1"][idx]),
                _pack_pvec(esc_v * inp["e_b2"][idx]),
            ], axis=-1)
        in_maps.append(m)

    from concourse.bass_utils import run_bass_kernel_spmd
    res = run_bass_kernel_spmd(nc, in_maps, list(range(NCORES)))

    h = np.empty((B, S, DIM), np.float32)
    dh = np.empty((B, S, DIM), np.float32)
    for c in range(NCORES):
        b, half = c // 2, c % 2
        tsl = slice(half * TLOC, (half + 1) * TLOC)
        h[b, tsl] = res.results[c]["out_e"].T
        dh[b, tsl] = res.results[c]["out_d"].T
    return h, dh
